# revision 8
# baseline (speedup 1.0000x reference)
"""Embedding lookup on 8 Trainium2 NeuronCores — bf16 bounce, eager stores.

out[b, s, :] = W[:, input[b, s]]   (W: [d_model, vocab])

Data-parallel over tokens (2048/core); host converts the table to bf16
(rel-err gate 2e-2, bf16 rounds within 0.4%) and upcasts the result, so the
device moves half the bytes each way. Per core: 16 SWDGE indirect gathers
of 128x2KB rows into SBUF. The Q7 descriptor-generation loop costs ~8ns per
descriptor (1.1us per 128-row op) and caps at one row per partition per op,
so the 16-op serial gen (~22.5us) is the critical path; the dma_gather ucode
alternative has the same per-row cost but adds a ~9.5us library load, so the
indirect path wins. Stores are per-tile (128x2KB descriptors) and issue the
moment each gather's completion sem fires, keeping writes flowing inside the
gen window's spare bus capacity and shrinking the post-gen drain tail to one
256KB store. Ramp trims: idx column 0 loads first so gather 0's generation
starts before the rest of the idx lands, and the unused partition-id caches
are prefilled so bass2jax's wrapper emits no per-engine TENSOR_LOADs at
program start.
"""
import sys

sys.path.insert(0, "/opt/trn_rl_repo")

import contextlib

import ml_dtypes
import numpy as np

import concourse.bass as bass
from concourse import mybir
from concourse.bass_utils import run_bass_kernel_spmd

VOCAB = 50257
D_MODEL = 1024
BATCH = 4
SEQ = 4096
N_CORES = 8
P = 128

TOKENS = BATCH * SEQ              # 16384
T_CORE = TOKENS // N_CORES        # 2048 tokens per core
NT = T_CORE // P                  # 16 gather ops of 128 rows

_compiled = None


def _build():
    # Strip the const-AP memsets + init-time all-engine barrier (this kernel
    # uses neither const_aps nor cross-engine state before its own sems), and
    # the monotonic-semaphore register machinery.
    orig_barrier = bass.Bass.all_engine_barrier
    orig_memset = bass.BassGpSimd.memset
    bass.Bass.all_engine_barrier = lambda self, **kw: None
    bass.BassGpSimd.memset = lambda self, *a, **kw: None
    try:
        nc = bass.Bass("TRN2", debug=False, num_devices=N_CORES,
                       monotonic_sem_count=0)
    finally:
        bass.Bass.all_engine_barrier = orig_barrier
        bass.BassGpSimd.memset = orig_memset
    table = nc.dram_tensor("table", [VOCAB, D_MODEL], mybir.dt.bfloat16,
                           kind="ExternalInput")
    idx = nc.dram_tensor("idx", [T_CORE], mybir.dt.int32, kind="ExternalInput")
    out = nc.dram_tensor("out", [T_CORE, D_MODEL], mybir.dt.bfloat16,
                         kind="ExternalOutput")

    with contextlib.ExitStack() as st:
        idx_tile = st.enter_context(nc.sbuf_tensor([P, NT], mybir.dt.int32))
        gbuf = st.enter_context(
            nc.sbuf_tensor([P, NT * D_MODEL], mybir.dt.bfloat16))
        idx_sem0 = st.enter_context(nc.semaphore("idx_sem0"))
        idx_sem1 = st.enter_context(nc.semaphore("idx_sem1"))
        g_sems = [st.enter_context(nc.semaphore(f"g{t}")) for t in range(NT)]
        s_sem = st.enter_context(nc.semaphore("s_sem"))
        block = st.enter_context(nc.Block())

        idx_v = idx.ap().rearrange("(p t) -> p t", p=P)
        # out rows p*NT + {2u, 2u+1} merge into one 4KB descriptor per
        # partition per store.
        out_v = out.ap().flatten().rearrange("(p u e) -> p u e", p=P, u=NT // 2)

        @block.sync
        def _(sync):
            sync.dma_start(idx_tile[:, 0:2], idx_v[:, 0:2]).then_inc(idx_sem0, 16)
            sync.dma_start(idx_tile[:, 2:NT], idx_v[:, 2:NT]).then_inc(idx_sem1, 16)
            for u in range(0, NT // 2, 2):
                sync.wait_ge(g_sems[2 * u], 16)
                sync.wait_ge(g_sems[2 * u + 1], 16)
                sync.dma_start(out_v[:, u, :],
                               gbuf[:, 2 * u * D_MODEL:(2 * u + 2) * D_MODEL]
                               ).then_inc(s_sem, 16)
            sync.wait_ge(s_sem, 16 * (NT // 2))

        @block.scalar
        def _(scalar):
            for u in range(1, NT // 2, 2):
                scalar.wait_ge(g_sems[2 * u], 16)
                scalar.wait_ge(g_sems[2 * u + 1], 16)
                scalar.dma_start(out_v[:, u, :],
                                 gbuf[:, 2 * u * D_MODEL:(2 * u + 2) * D_MODEL]
                                 ).then_inc(s_sem, 16)

        @block.gpsimd
        def _(gpsimd):
            gpsimd.wait_ge(idx_sem0, 16)
            for t in range(NT):
                if t == 2:
                    gpsimd.wait_ge(idx_sem1, 16)
                gpsimd.indirect_dma_start(
                    out=gbuf[:, t * D_MODEL:(t + 1) * D_MODEL],
                    out_offset=None,
                    in_=table.ap(),
                    in_offset=bass.IndirectOffsetOnAxis(
                        ap=idx_tile[:, t:t + 1], axis=0),
                ).then_inc(g_sems[t], 16)

    # Nothing here reads partition_id; prefill the caches so bass2jax's
    # cache_partition_id() emits no per-engine TENSOR_LOADs at program start.
    for eng in nc.engines.values():
        if eng._cached_partition_id is None:
            eng._cached_partition_id = 0
    nc._cached_partition_id_multi[tuple(mybir.ALL_ENGINES)] = 0
    return nc


def prep_in_maps(input: np.ndarray, W: np.ndarray):
    table_np = np.ascontiguousarray(
        np.asarray(W, dtype=np.float32).T.astype(ml_dtypes.bfloat16))
    idx_flat = np.ascontiguousarray(
        np.asarray(input, dtype=np.int32).reshape(TOKENS))
    return [
        {"table": table_np, "idx": idx_flat[k * T_CORE:(k + 1) * T_CORE]}
        for k in range(N_CORES)
    ]


def kernel(input: np.ndarray, W: np.ndarray) -> np.ndarray:
    global _compiled
    assert input.shape == (BATCH, SEQ) and W.shape == (D_MODEL, VOCAB)
    if _compiled is None:
        _compiled = _build()
    nc = _compiled

    in_maps = prep_in_maps(input, W)
    res = run_bass_kernel_spmd(nc, in_maps, core_ids=list(range(N_CORES)))
    out = np.concatenate(
        [np.asarray(res.results[k]["out"]) for k in range(N_CORES)], axis=0)
    return out.astype(np.float32).reshape(BATCH, SEQ, D_MODEL)


# revision 9
# speedup vs baseline: 1.0022x; 1.0022x over previous
"""Embedding lookup on 8 Trainium2 NeuronCores — bf16 bounce, eager stores.

out[b, s, :] = W[:, input[b, s]]   (W: [d_model, vocab])

Data-parallel over tokens (2048/core); host converts the table to bf16
(rel-err gate 2e-2, bf16 rounds within 0.4%) and upcasts the result, so the
device moves half the bytes each way. Per core: 16 SWDGE indirect gathers
of 128x2KB rows into SBUF. The Q7 descriptor-generation loop costs ~8ns per
descriptor (1.1us per 128-row op) and caps at one row per partition per op,
so the 16-op serial gen (~22.5us) is the critical path; the dma_gather ucode
alternative has the same per-row cost but adds a ~9.5us library load, so the
indirect path wins. Stores are per-tile (128x2KB descriptors), issue the
moment each gather's completion sem fires, and alternate between the SP and
Activation HWDGE queues so two engines dispatch/drain writes in parallel —
keeping writes flowing inside the gen window's spare bus capacity and
shrinking the post-gen drain tail to one 256KB store. Ramp trims: idx column 0 loads first so gather 0's generation
starts before the rest of the idx lands, and the unused partition-id caches
are prefilled so bass2jax's wrapper emits no per-engine TENSOR_LOADs at
program start.
"""
import sys

sys.path.insert(0, "/opt/trn_rl_repo")

import contextlib

import ml_dtypes
import numpy as np

import concourse.bass as bass
from concourse import mybir
from concourse.bass_utils import run_bass_kernel_spmd

VOCAB = 50257
D_MODEL = 1024
BATCH = 4
SEQ = 4096
N_CORES = 8
P = 128

TOKENS = BATCH * SEQ              # 16384
T_CORE = TOKENS // N_CORES        # 2048 tokens per core
NT = T_CORE // P                  # 16 gather ops of 128 rows

_compiled = None


def _build():
    # Strip the const-AP memsets + init-time all-engine barrier (this kernel
    # uses neither const_aps nor cross-engine state before its own sems), and
    # the monotonic-semaphore register machinery.
    orig_barrier = bass.Bass.all_engine_barrier
    orig_memset = bass.BassGpSimd.memset
    bass.Bass.all_engine_barrier = lambda self, **kw: None
    bass.BassGpSimd.memset = lambda self, *a, **kw: None
    try:
        nc = bass.Bass("TRN2", debug=False, num_devices=N_CORES,
                       monotonic_sem_count=0)
    finally:
        bass.Bass.all_engine_barrier = orig_barrier
        bass.BassGpSimd.memset = orig_memset
    table = nc.dram_tensor("table", [VOCAB, D_MODEL], mybir.dt.bfloat16,
                           kind="ExternalInput")
    idx = nc.dram_tensor("idx", [T_CORE], mybir.dt.int32, kind="ExternalInput")
    out = nc.dram_tensor("out", [T_CORE, D_MODEL], mybir.dt.bfloat16,
                         kind="ExternalOutput")

    with contextlib.ExitStack() as st:
        idx_tile = st.enter_context(nc.sbuf_tensor([P, NT], mybir.dt.int32))
        gbuf = st.enter_context(
            nc.sbuf_tensor([P, NT * D_MODEL], mybir.dt.bfloat16))
        idx_sem0 = st.enter_context(nc.semaphore("idx_sem0"))
        idx_sem1 = st.enter_context(nc.semaphore("idx_sem1"))
        g_sems = [st.enter_context(nc.semaphore(f"g{t}")) for t in range(NT)]
        s_sem = st.enter_context(nc.semaphore("s_sem"))
        block = st.enter_context(nc.Block())

        idx_v = idx.ap().rearrange("(p t) -> p t", p=P)
        # out row p*NT + t is one 2KB descriptor per partition per store.
        out_v = out.ap().flatten().rearrange("(p t e) -> p t e", p=P, t=NT)

        @block.sync
        def _(sync):
            sync.dma_start(idx_tile[:, 0:2], idx_v[:, 0:2]).then_inc(idx_sem0, 16)
            sync.dma_start(idx_tile[:, 2:NT], idx_v[:, 2:NT]).then_inc(idx_sem1, 16)
            for t in range(0, NT, 2):
                sync.wait_ge(g_sems[t], 16)
                sync.dma_start(out_v[:, t, :],
                               gbuf[:, t * D_MODEL:(t + 1) * D_MODEL]
                               ).then_inc(s_sem, 16)
            sync.wait_ge(s_sem, 16 * NT)

        @block.scalar
        def _(scalar):
            for t in range(1, NT, 2):
                scalar.wait_ge(g_sems[t], 16)
                scalar.dma_start(out_v[:, t, :],
                                 gbuf[:, t * D_MODEL:(t + 1) * D_MODEL]
                                 ).then_inc(s_sem, 16)

        @block.gpsimd
        def _(gpsimd):
            gpsimd.wait_ge(idx_sem0, 16)
            for t in range(NT):
                if t == 2:
                    gpsimd.wait_ge(idx_sem1, 16)
                gpsimd.indirect_dma_start(
                    out=gbuf[:, t * D_MODEL:(t + 1) * D_MODEL],
                    out_offset=None,
                    in_=table.ap(),
                    in_offset=bass.IndirectOffsetOnAxis(
                        ap=idx_tile[:, t:t + 1], axis=0),
                ).then_inc(g_sems[t], 16)

    # Nothing here reads partition_id; prefill the caches so bass2jax's
    # cache_partition_id() emits no per-engine TENSOR_LOADs at program start.
    for eng in nc.engines.values():
        if eng._cached_partition_id is None:
            eng._cached_partition_id = 0
    nc._cached_partition_id_multi[tuple(mybir.ALL_ENGINES)] = 0
    return nc


def prep_in_maps(input: np.ndarray, W: np.ndarray):
    table_np = np.ascontiguousarray(
        np.asarray(W, dtype=np.float32).T.astype(ml_dtypes.bfloat16))
    idx_flat = np.ascontiguousarray(
        np.asarray(input, dtype=np.int32).reshape(TOKENS))
    return [
        {"table": table_np, "idx": idx_flat[k * T_CORE:(k + 1) * T_CORE]}
        for k in range(N_CORES)
    ]


def kernel(input: np.ndarray, W: np.ndarray) -> np.ndarray:
    global _compiled
    assert input.shape == (BATCH, SEQ) and W.shape == (D_MODEL, VOCAB)
    if _compiled is None:
        _compiled = _build()
    nc = _compiled

    in_maps = prep_in_maps(input, W)
    res = run_bass_kernel_spmd(nc, in_maps, core_ids=list(range(N_CORES)))
    out = np.concatenate(
        [np.asarray(res.results[k]["out"]) for k in range(N_CORES)], axis=0)
    return out.astype(np.float32).reshape(BATCH, SEQ, D_MODEL)
